# revision 7
# baseline (speedup 1.0000x reference)
"""Causal self-attention (B=4, T=2048, D=1024, H=16) on 8 trn2 NeuronCores.

Sharding: batch (4-way) x head-half (2-way tensor parallel) => 8 cores,
one uniform SPMD program (per-core differences are pure data: which batch's
x, which half of the QKV columns / proj columns each core receives).

v2 layout: the QKV projection, attention, AllGather and proj phases are
interleaved at query-tile granularity so every engine has work throughout
the kernel (v1 ran them as four sequential phases; the PE idled during the
ScalarE-bound softmax phase long enough for the HAM clock gate to hold the
array at half clock for ~250us of the run).

Per core (batch b, head-half hh, 8 local heads), all matmul operands bf16:
  for tt in 0..3 (token/query tiles of 512):
    1. QKV(tt): q^T/k^T in [qkv_col, token] layout; v in [token, vcol+1]
       layout (ones column => PV matmul accumulates the softmax denominator
       l in PSUM row HD).
    2. attention(qt=tt): per head-partition-group, stream key-block PAIRS
       (2x128): S^T matmuls (2 heads concurrent in disjoint PE row groups)
       -> one merged exp per pair on ScalarE (off-diagonal pairs N=1024)
       -> PV deferred one pair (software pipeline) so the PE never waits
       on the exp latency.
    3. normalization: l rows of all 8 heads batched into one DVE
       reciprocal, broadcast across partitions via a K=8 esel matmul.
    4. ship y^T: pairwise AllGather (bf16) with the partner core; overlaps
       the next tile's QKV matmuls.
    5. proj(qt-1): column-sharded, emitted one tile behind so the gather
       has a full QKV+attention window to land.
"""

import os
import sys
from dataclasses import dataclass

import ml_dtypes
import numpy as np

sys.path.insert(0, "/opt/trn_rl_repo")

import concourse.mybir as mybir  # noqa: E402
import concourse.tile as tile  # noqa: E402
from concourse import bacc  # noqa: E402
from concourse.bass import ds, ts  # noqa: E402

P = 128
F32 = mybir.dt.float32
BF16 = mybir.dt.bfloat16
AF = mybir.ActivationFunctionType
ALU = mybir.AluOpType
BF16NP = ml_dtypes.bfloat16


@dataclass(frozen=True)
class Cfg:
    T: int = 2048          # sequence length
    D: int = 1024          # model dim (QKV contraction dim)
    H_LOC: int = 8         # heads per core
    HD: int = 64           # head dim
    TT: int = 512          # token tile width in the QKV phase
    QT: int = 512          # query tile width in the attention phase
    n_groups: int = 2      # cores sharing a batch (pairwise AllGather)
    scale: float = 64 ** -0.5

    @property
    def DH(self):          # local head dims (y^T rows contributed per core)
        return self.H_LOC * self.HD

    @property
    def GDH(self):         # proj contraction dim (= model dim)
        return self.n_groups * self.DH

    @property
    def DCH(self):
        return self.D // P

    @property
    def NHP(self):         # 128-partition groups of local head dims
        return self.DH // P

    @property
    def HPG(self):         # heads per 128-partition group
        return P // self.HD

    @property
    def NTT(self):
        return self.T // self.TT

    @property
    def NQT(self):
        return self.T // self.QT

    @property
    def CB(self):          # 128-wide column blocks of the local q/k cols
        return self.DH // P


FULL = Cfg()


def build_nc(c: Cfg, n_cores: int = 8, with_bias: bool = True):
    """Build the (uniform SPMD) Bass program for one core."""
    assert c.T % c.TT == 0 and c.T % c.QT == 0 and c.QT % P == 0
    assert c.D % P == 0 and c.DH % P == 0 and c.TT % P == 0
    assert c.TT == c.QT, "QKV token tiles and query tiles must align"
    use_cc = c.n_groups > 1

    nc = bacc.Bacc(
        "TRN2", target_bir_lowering=False, debug=False, num_devices=n_cores
    )
    xT = nc.dram_tensor("xT", [c.D, c.T], BF16, kind="ExternalInput").ap()
    wq = nc.dram_tensor("wq", [c.D, c.DH], BF16, kind="ExternalInput").ap()
    wk = nc.dram_tensor("wk", [c.D, c.DH], BF16, kind="ExternalInput").ap()
    wv = nc.dram_tensor("wv", [c.D, c.DH], BF16, kind="ExternalInput").ap()
    bq = nc.dram_tensor("bq", [c.DH], F32, kind="ExternalInput").ap()
    bk = nc.dram_tensor("bk", [c.DH], F32, kind="ExternalInput").ap()
    bv = nc.dram_tensor("bv", [1, c.DH], BF16, kind="ExternalInput").ap()
    wp = nc.dram_tensor("wp", [c.GDH, c.DH], BF16, kind="ExternalInput").ap()
    bp = nc.dram_tensor("bp", [1, c.DH], BF16, kind="ExternalInput").ap()
    oc = max(P, (c.T // P) * c.H_LOC)
    onesin = nc.dram_tensor("onesin", [P, oc], BF16, kind="ExternalInput").ap()
    esel = nc.dram_tensor("esel", [c.H_LOC, c.NHP * P], BF16,
                          kind="ExternalInput").ap()
    out = nc.dram_tensor("out", [c.T, c.DH], F32, kind="ExternalOutput").ap()

    groups = [[g * c.n_groups + i for i in range(c.n_groups)]
              for g in range(max(1, n_cores // c.n_groups))]

    with tile.TileContext(nc) as tc:
        with (
            tc.tile_pool(name="const", bufs=1) as cst,
            tc.tile_pool(name="kv", bufs=1) as kv,
            tc.tile_pool(name="wqkv", bufs=1) as wqk,
            tc.tile_pool(name="xt", bufs=2) as xtp,
            tc.tile_pool(name="wproj", bufs=1) as wpp,
            tc.tile_pool(name="pt", bufs=4) as ptp,
            tc.tile_pool(name="yt", bufs=2) as ytp,
            tc.tile_pool(name="yu", bufs=1) as yup,
            tc.tile_pool(name="lr", bufs=2) as lrp,
            tc.tile_pool(name="yag", bufs=2) as yagp,
            tc.tile_pool(name="osb", bufs=2) as osbp,
            tc.tile_pool(name="ps_mm", bufs=2, space="PSUM") as ps_mm,
            tc.tile_pool(name="ps_s", bufs=2, space="PSUM") as ps_s,
            tc.tile_pool(name="ps_y", bufs=2, space="PSUM") as ps_y,
            tc.tile_pool(name="dram", bufs=2, space="DRAM") as drp,
        ):
            # ---- first input tile + weights (DMA-order: critical first) ----
            xT_r = xT.rearrange("(ch p) t -> p ch t", p=P)
            xts = [None] * c.NTT
            xts[0] = xtp.tile([P, c.DCH, c.TT], BF16, name="xt0")
            nc.sync.dma_start(xts[0], xT_r[:, :, ts(0, c.TT)])

            wq_sb = wqk.tile([P, c.DCH, c.DH], BF16)
            wk_sb = wqk.tile([P, c.DCH, c.DH], BF16)
            wv_sb = wqk.tile([P, c.DCH, c.DH], BF16)
            wr = {"wq": wq.rearrange("(ch p) n -> p ch n", p=P),
                  "wk": wk.rearrange("(ch p) n -> p ch n", p=P),
                  "wv": wv.rearrange("(ch p) n -> p ch n", p=P)}
            for dc in range(c.DCH):
                nc.gpsimd.dma_start(wk_sb[:, dc, :], wr["wk"][:, dc, :])
                nc.scalar.dma_start(wq_sb[:, dc, :], wr["wq"][:, dc, :])
                nc.scalar.dma_start(wv_sb[:, dc, :], wr["wv"][:, dc, :])

            # ---- constants ----
            ones_row = cst.tile([1, P], BF16)
            nc.gpsimd.dma_start(ones_row, onesin[0:1, 0:P])
            bq_sb = cst.tile([P, c.CB], F32)
            nc.scalar.dma_start(bq_sb, bq.rearrange("(cb p) -> p cb", p=P))
            bk_sb = cst.tile([P, c.CB], F32)
            nc.scalar.dma_start(bk_sb, bk.rearrange("(cb p) -> p cb", p=P))
            bv_row = cst.tile([1, c.DH], BF16)
            nc.scalar.dma_start(bv_row, bv)
            bp_row = cst.tile([1, c.DH], BF16)
            nc.scalar.dma_start(bp_row, bp)
            esel_sb = cst.tile([c.H_LOC, c.NHP * P], BF16)
            nc.gpsimd.dma_start(esel_sb, esel)
            # causal triangle for the key block AT the diagonal:
            # mask[k, j] keeps where j - k >= 0 (j = query col within window)
            mask = cst.tile([P, P], BF16)
            nc.vector.memset(mask, 1.0)
            nc.gpsimd.affine_select(
                mask, mask,
                compare_op=ALU.is_ge, fill=0.0, base=0,
                pattern=[[1, P]], channel_multiplier=-1,
            )

            # ---- persistent K^T / Q^T / V(+ones) ----
            kT = kv.tile([P, c.NHP, c.T], BF16)
            qT = kv.tile([P, c.NHP, c.T], BF16)
            v = kv.tile([P, c.T // P, c.H_LOC, c.HD + 1], BF16)
            nc.vector.memset(v[:, :, :, c.HD:c.HD + 1], 1.0)

            wp_sb = wpp.tile([P, c.GDH // P, c.DH], BF16)

            def emit_qkv(tt):
                xt = xts[tt]
                # K^T and Q^T: [col, token] layout
                for dst, w_sb, b_sb in (
                    (kT, wk_sb, bk_sb),
                    (qT, wq_sb, bq_sb),
                ):
                    for cb in range(c.CB):
                        pst = ps_mm.tile([P, max(c.TT, c.DH)], F32,
                                         tag="mm", name="pst")[:, :c.TT]
                        for dc in range(c.DCH):
                            nc.tensor.matmul(
                                pst,
                                w_sb[:, dc, ts(cb, P)],
                                xt[:, dc, :],
                                start=(dc == 0),
                                stop=(dc == c.DCH - 1),
                            )
                        if with_bias:
                            nc.vector.tensor_tensor(
                                dst[:, cb, ts(tt, c.TT)], pst,
                                b_sb[:, cb:cb + 1].to_broadcast((P, c.TT)),
                                ALU.add,
                            )
                        else:
                            nc.vector.tensor_copy(
                                dst[:, cb, ts(tt, c.TT)], pst)

                # V: [token, vcol] layout (+ bias via ones-row matmul)
                for tb in range(c.TT // P):
                    gtb = tt * (c.TT // P) + tb
                    psv = ps_mm.tile([P, max(c.TT, c.DH)], F32,
                                     tag="mm", name="psv")[:, :c.DH]
                    for dc in range(c.DCH):
                        nc.tensor.matmul(
                            psv,
                            xt[:, dc, ts(tb, P)],
                            wv_sb[:, dc, :],
                            start=(dc == 0),
                            stop=(not with_bias and dc == c.DCH - 1),
                        )
                    if with_bias:
                        nc.tensor.matmul(
                            psv, ones_row[0:1, 0:P], bv_row,
                            start=False, stop=True,
                        )
                    nc.vector.tensor_copy(
                        v[:, gtb, :, 0:c.HD],
                        psv.rearrange("p (h d) -> p h d", d=c.HD),
                    )

            def emit_attn(qt, yt_q, l_all):
                """Attention for query tile qt -> yt_q (normalized, bf16)."""
                yu_q = yup.tile([P, c.NHP, c.QT], F32)
                nkb = (qt + 1) * c.QT // P
                for hp in range(c.NHP):
                    psys = [ps_y.tile([c.HD + 1, c.QT], F32, tag="psy",
                                      name=f"psy{hs}")
                            for hs in range(c.HPG)]

                    def emit_pv(kb2, blocks, pts):
                        for j in range(blocks):
                            kb = kb2 + j
                            off = max(0, kb * P - qt * c.QT)
                            for hs in range(c.HPG):
                                nc.tensor.matmul(
                                    psys[hs][:, off:],
                                    v[:, kb, hp * c.HPG + hs, :],
                                    pts[hs][:, j, off:],
                                    start=(kb == 0),
                                    stop=(kb == nkb - 1),
                                )

                    pending = None
                    for kb2 in range(0, nkb, 2):
                        blocks = min(2, nkb - kb2)
                        # S^T matmuls: the HPG heads sharing this partition
                        # group run in disjoint PE row groups (tile_position
                        # auto-derived from base_partition) => concurrent.
                        pss_l = []
                        for hs in range(c.HPG):
                            pb = hs * c.HD
                            pss = ps_s.tile([P, 2, c.QT], F32, tag="pss",
                                            name=f"pss{hs}")
                            for j in range(blocks):
                                off = max(0, (kb2 + j) * P - qt * c.QT)
                                nc.tensor.matmul(
                                    pss[:, j, off:],
                                    kT[pb:pb + c.HD, hp, ts(kb2 + j, P)],
                                    qT[pb:pb + c.HD, hp,
                                       ds(qt * c.QT + off, c.QT - off)],
                                    start=True, stop=True,
                                )
                            pss_l.append(pss)
                        # PV of the previous pair (software pipeline: the PE
                        # issues these while ScalarE exps the current pair)
                        if pending is not None:
                            emit_pv(*pending)
                        # exp (+ causal triangle on the diagonal blocks)
                        pt_l = []
                        for hs in range(c.HPG):
                            pt = ptp.tile([P, 2, c.QT], BF16,
                                          tag="pt", name=f"pt{hs}")
                            off0 = kb2 * P - qt * c.QT
                            if off0 < 0 and blocks == 2:
                                # fully-unmasked pair: one merged exp N=2*QT
                                nc.scalar.activation(
                                    pt.rearrange("p j q -> p (j q)"),
                                    pss_l[hs].rearrange("p j q -> p (j q)"),
                                    AF.Exp, scale=c.scale)
                            else:
                                for j in range(blocks):
                                    off = max(0, (kb2 + j) * P - qt * c.QT)
                                    nc.scalar.activation(
                                        pt[:, j, off:],
                                        pss_l[hs][:, j, off:],
                                        AF.Exp, scale=c.scale)
                            for j in range(blocks):
                                off = (kb2 + j) * P - qt * c.QT
                                if off >= 0:
                                    nc.vector.tensor_tensor(
                                        pt[:, j, off:off + P],
                                        pt[:, j, off:off + P],
                                        mask, ALU.mult,
                                    )
                            pt_l.append(pt)
                        pending = (kb2, blocks, pt_l)
                    emit_pv(*pending)

                    # stage unnormalized y^T + the denominators (direct
                    # partition-shifted DVE copies out of PSUM)
                    for hs in range(c.HPG):
                        pb = hs * c.HD
                        nc.vector.tensor_copy(
                            yu_q[pb:pb + c.HD, hp, :],
                            psys[hs][0:c.HD, :])
                        row = hp * c.HPG + hs
                        l_sb = lrp.tile([1, c.QT], F32, tag="lsb",
                                        name="l_sb")
                        nc.vector.tensor_copy(
                            l_sb, psys[hs][c.HD:c.HD + 1, :])
                        nc.gpsimd.dma_start(
                            l_all[row:row + 1, :], l_sb)

                # batched softmax normalization for all 8 heads
                r_all = lrp.tile([c.H_LOC, c.QT], BF16, tag="rall")
                with nc.allow_low_precision(
                    reason="1/l rounded to bf16 for the broadcast matmul"
                ):
                    nc.vector.reciprocal(r_all, l_all)
                for hp in range(c.NHP):
                    psr = ps_mm.tile([P, max(c.TT, c.DH)], F32,
                                     tag="mm", name="psr")[:, :c.QT]
                    nc.tensor.matmul(
                        psr, esel_sb[:, ts(hp, P)], r_all,
                        start=True, stop=True,
                    )
                    nc.vector.tensor_tensor(
                        yt_q[:, hp, :], yu_q[:, hp, :], psr, ALU.mult,
                    )

            def emit_gather(qt, yt_q, halves):
                """Ship y^T; pairwise AllGather along the dims axis."""
                hw_ = c.QT // halves
                y_ags = []
                for hf in range(halves):
                    tsl = ds(hf * hw_, hw_)
                    y_loc = drp.tile([c.DH, hw_], BF16,
                                     tag=f"yloc{halves}", name="y_loc")
                    nc.sync.dma_start(
                        y_loc.rearrange("(hp p) t -> p hp t", p=P),
                        yt_q[:, :, tsl],
                    )
                    if use_cc:
                        y_ag = drp.tile([c.GDH, hw_], BF16,
                                        tag=f"ygat{halves}", name="y_ag")
                        nc.gpsimd.collective_compute(
                            "AllGather", ALU.bypass,
                            replica_groups=groups,
                            ins=[y_loc.opt()], outs=[y_ag.opt()],
                        )
                    else:
                        y_ag = y_loc
                    y_ags.append(y_ag)
                return y_ags

            def emit_proj(qt, y_ags, halves):
                """Column-sharded proj for query tile qt."""
                hw_ = c.QT // halves
                for hf in range(halves):
                    yag_sb = yagp.tile([P, c.GDH // P, c.QT], BF16,
                                       name="yag_sb")[:, :, :hw_]
                    nc.sync.dma_start(
                        yag_sb,
                        y_ags[hf].rearrange("(ch p) t -> p ch t", p=P),
                    )
                    for tb in range(hw_ // P):
                        gtb = hf * (hw_ // P) + tb
                        pso = ps_mm.tile([P, max(c.TT, c.DH)], F32,
                                         tag="mm", name="pso")[:, :c.DH]
                        for c2 in range(c.GDH // P):
                            nc.tensor.matmul(
                                pso,
                                yag_sb[:, c2, ts(tb, P)],
                                wp_sb[:, c2, :],
                                start=(c2 == 0),
                                stop=(not with_bias
                                      and c2 == c.GDH // P - 1),
                            )
                        if with_bias:
                            nc.tensor.matmul(
                                pso, ones_row[0:1, 0:P], bp_row,
                                start=False, stop=True,
                            )
                        osb = osbp.tile([P, c.DH], F32)
                        nc.vector.tensor_copy(osb, pso)
                        nc.gpsimd.dma_start(
                            out[ds(qt * c.QT + gtb * P, P), :], osb)

            # ================= main interleaved loop =================
            pending_proj = None
            for tt in range(c.NTT):
                if tt + 1 < c.NTT:
                    xts[tt + 1] = xtp.tile([P, c.DCH, c.TT], BF16,
                                           name=f"xt{tt + 1}")
                    nc.sync.dma_start(
                        xts[tt + 1], xT_r[:, :, ts(tt + 1, c.TT)])
                emit_qkv(tt)
                if tt == 0:
                    # proj weights: deferred so the startup DMA burst serves
                    # x/wq/wk/wv first
                    nc.gpsimd.dma_start(
                        wp_sb, wp.rearrange("(ch p) n -> p ch n", p=P))
                yt_q = ytp.tile([P, c.NHP, c.QT], BF16)
                l_all = lrp.tile([c.H_LOC, c.QT], F32, tag="lall")
                emit_attn(tt, yt_q, l_all)
                halves = 2 if (tt == c.NQT - 1 and c.QT // 2 >= P) else 1
                y_ags = emit_gather(tt, yt_q, halves)
                if pending_proj is not None:
                    emit_proj(*pending_proj)
                pending_proj = (tt, y_ags, halves)
            emit_proj(*pending_proj)

    nc.compile()
    return nc


def shard_inputs(c: Cfg, x, w_qkv, b_qkv, w_proj, b_proj, n_cores=8):
    """Full fp32 inputs -> per-core input maps (host-side marshalling).

    Matmul operands are cast to bf16 on the host; q/k biases stay fp32
    (applied via DVE adds on the f32 PSUM)."""
    D, DH = c.D, c.DH
    oc = max(128, (c.T // 128) * c.H_LOC)
    ones = np.ones((128, oc), BF16NP)
    esel = np.zeros((c.H_LOC, c.NHP * 128), BF16NP)
    for h in range(c.H_LOC):
        hp, sub = h // c.HPG, h % c.HPG
        esel[h, hp * 128 + sub * c.HD: hp * 128 + (sub + 1) * c.HD] = 1
    maps = []
    for core in range(n_cores):
        b, hh = core // c.n_groups, core % c.n_groups
        sl = slice(hh * DH, (hh + 1) * DH)
        maps.append({
            "xT": np.ascontiguousarray(x[b].T).astype(BF16NP),
            "wq": np.ascontiguousarray(
                w_qkv[:, 0 * D:1 * D][:, sl]).astype(BF16NP),
            "wk": np.ascontiguousarray(
                w_qkv[:, 1 * D:2 * D][:, sl]).astype(BF16NP),
            "wv": np.ascontiguousarray(
                w_qkv[:, 2 * D:3 * D][:, sl]).astype(BF16NP),
            "bq": np.ascontiguousarray(
                b_qkv[0 * D:1 * D][sl], dtype=np.float32),
            "bk": np.ascontiguousarray(
                b_qkv[1 * D:2 * D][sl], dtype=np.float32),
            "bv": np.ascontiguousarray(
                b_qkv[2 * D:3 * D][sl]).reshape(1, DH).astype(BF16NP),
            "wp": np.ascontiguousarray(w_proj[:, sl]).astype(BF16NP),
            "bp": np.ascontiguousarray(
                b_proj[sl]).reshape(1, DH).astype(BF16NP),
            "onesin": ones,
            "esel": esel,
        })
    return maps


def gather_outputs(c: Cfg, results, n_cores=8):
    B = n_cores // c.n_groups
    out = np.empty((B, c.T, c.GDH), dtype=np.float32)
    for core in range(n_cores):
        b, hh = core // c.n_groups, core % c.n_groups
        out[b][:, hh * c.DH:(hh + 1) * c.DH] = results[core]["out"]
    return out


_NC_CACHE: dict = {}


def kernel(**inputs) -> np.ndarray:
    from concourse.bass_utils import run_bass_kernel_spmd

    c = FULL
    n_cores = 8
    wb = bool(np.any(inputs["b_qkv"]) or np.any(inputs["b_proj"]))
    key = (c, n_cores, wb)
    if key not in _NC_CACHE:
        _NC_CACHE[key] = build_nc(c, n_cores, with_bias=wb)
    nc = _NC_CACHE[key]
    in_maps = shard_inputs(
        c, inputs["x"], inputs["w_qkv"], inputs["b_qkv"],
        inputs["w_proj"], inputs["b_proj"], n_cores,
    )
    res = run_bass_kernel_spmd(
        nc, in_maps, core_ids=list(range(n_cores)),
        trace=bool(int(os.environ.get("KERNEL_TRACE", "0"))),
    )
    kernel.last_results = res
    return gather_outputs(c, res.results, n_cores)


# revision 9
# speedup vs baseline: 1.1534x; 1.1534x over previous
"""Causal self-attention (B=4, T=2048, D=1024, H=16) on 8 trn2 NeuronCores.

Sharding: batch (4-way) x head-half (2-way tensor parallel) => 8 cores,
one uniform SPMD program (per-core differences are pure data: which batch's
x, which half of the QKV columns / proj columns each core receives).

v2 layout: the QKV projection, attention, AllGather and proj phases are
interleaved at query-tile granularity so every engine has work throughout
the kernel (v1 ran them as four sequential phases; the PE idled during the
ScalarE-bound softmax phase long enough for the HAM clock gate to hold the
array at half clock for ~250us of the run).

Per core (batch b, head-half hh, 8 local heads), all matmul operands bf16:
  for tt in 0..3 (token/query tiles of 512):
    1. QKV(tt): q^T/k^T in [qkv_col, token] layout; v in [token, vcol+1]
       layout (ones column => PV matmul accumulates the softmax denominator
       l in PSUM row HD).
    2. attention(qt=tt): per head-partition-group, stream key-block PAIRS
       (2x128): S^T matmuls (2 heads concurrent in disjoint PE row groups)
       -> one merged exp per pair on ScalarE (off-diagonal pairs N=1024)
       -> PV deferred one pair (software pipeline) so the PE never waits
       on the exp latency.
    3. normalization: l rows of all 8 heads batched into one DVE
       reciprocal, broadcast across partitions via a K=8 esel matmul.
    4. ship y^T: pairwise AllGather (bf16) with the partner core; overlaps
       the next tile's QKV matmuls.
    5. proj(qt-1): column-sharded, emitted one tile behind so the gather
       has a full QKV+attention window to land.
"""

import os
import sys
from dataclasses import dataclass

import ml_dtypes
import numpy as np

sys.path.insert(0, "/opt/trn_rl_repo")

import concourse.mybir as mybir  # noqa: E402
import concourse.tile as tile  # noqa: E402
from concourse import bacc  # noqa: E402
from concourse.bass import ds, ts  # noqa: E402

P = 128
F32 = mybir.dt.float32
BF16 = mybir.dt.bfloat16
AF = mybir.ActivationFunctionType
ALU = mybir.AluOpType
BF16NP = ml_dtypes.bfloat16


@dataclass(frozen=True)
class Cfg:
    T: int = 2048          # sequence length
    D: int = 1024          # model dim (QKV contraction dim)
    H_LOC: int = 8         # heads per core
    HD: int = 64           # head dim
    TT: int = 512          # token tile width in the QKV phase
    QT: int = 512          # query tile width in the attention phase
    n_groups: int = 2      # cores sharing a batch (pairwise AllGather)
    scale: float = 64 ** -0.5

    @property
    def DH(self):          # local head dims (y^T rows contributed per core)
        return self.H_LOC * self.HD

    @property
    def GDH(self):         # proj contraction dim (= model dim)
        return self.n_groups * self.DH

    @property
    def DCH(self):
        return self.D // P

    @property
    def NHP(self):         # 128-partition groups of local head dims
        return self.DH // P

    @property
    def HPG(self):         # heads per 128-partition group
        return P // self.HD

    @property
    def NTT(self):
        return self.T // self.TT

    @property
    def NQT(self):
        return self.T // self.QT

    @property
    def CB(self):          # 128-wide column blocks of the local q/k cols
        return self.DH // P


FULL = Cfg()


def build_nc(c: Cfg, n_cores: int = 8, with_bias: bool = True):
    """Build the (uniform SPMD) Bass program for one core."""
    assert c.T % c.TT == 0 and c.T % c.QT == 0 and c.QT % P == 0
    assert c.D % P == 0 and c.DH % P == 0 and c.TT % P == 0
    assert c.TT == c.QT, "QKV token tiles and query tiles must align"
    use_cc = c.n_groups > 1

    nc = bacc.Bacc(
        "TRN2", target_bir_lowering=False, debug=False, num_devices=n_cores
    )
    xT = nc.dram_tensor("xT", [c.D, c.T], BF16, kind="ExternalInput").ap()
    wq = nc.dram_tensor("wq", [c.D, c.DH], BF16, kind="ExternalInput").ap()
    wk = nc.dram_tensor("wk", [c.D, c.DH], BF16, kind="ExternalInput").ap()
    wv = nc.dram_tensor("wv", [c.D, c.DH], BF16, kind="ExternalInput").ap()
    bq = nc.dram_tensor("bq", [c.DH], F32, kind="ExternalInput").ap()
    bk = nc.dram_tensor("bk", [c.DH], F32, kind="ExternalInput").ap()
    bv = nc.dram_tensor("bv", [1, c.DH], BF16, kind="ExternalInput").ap()
    wp = nc.dram_tensor("wp", [c.GDH, c.DH], BF16, kind="ExternalInput").ap()
    bp = nc.dram_tensor("bp", [1, c.DH], BF16, kind="ExternalInput").ap()
    oc = max(P, (c.T // P) * c.H_LOC)
    onesin = nc.dram_tensor("onesin", [P, oc], BF16, kind="ExternalInput").ap()
    esel = nc.dram_tensor("esel", [c.H_LOC, c.NHP * P], BF16,
                          kind="ExternalInput").ap()
    out = nc.dram_tensor("out", [c.T, c.DH], F32, kind="ExternalOutput").ap()

    groups = [[g * c.n_groups + i for i in range(c.n_groups)]
              for g in range(max(1, n_cores // c.n_groups))]

    with tile.TileContext(nc) as tc:
        with (
            tc.tile_pool(name="const", bufs=1) as cst,
            tc.tile_pool(name="kv", bufs=1) as kv,
            tc.tile_pool(name="wqkv", bufs=1) as wqk,
            tc.tile_pool(name="xt", bufs=2) as xtp,
            tc.tile_pool(name="wproj", bufs=1) as wpp,
            tc.tile_pool(name="pt", bufs=4) as ptp,
            tc.tile_pool(name="yt", bufs=2) as ytp,
            tc.tile_pool(name="yu", bufs=1) as yup,
            tc.tile_pool(name="lr", bufs=2) as lrp,
            tc.tile_pool(name="yag", bufs=2) as yagp,
            tc.tile_pool(name="osb", bufs=2) as osbp,
            tc.tile_pool(name="ps_mm", bufs=2, space="PSUM") as ps_mm,
            tc.tile_pool(name="ps_s", bufs=2, space="PSUM") as ps_s,
            tc.tile_pool(name="ps_y", bufs=2, space="PSUM") as ps_y,
            tc.tile_pool(name="dram", bufs=2, space="DRAM") as drp,
        ):
            # ---- first input tile + weights (DMA-order: critical first) ----
            xT_r = xT.rearrange("(ch p) t -> p ch t", p=P)
            xts = [None] * c.NTT
            xts[0] = xtp.tile([P, c.DCH, c.TT], BF16, name="xt0")
            nc.sync.dma_start(xts[0], xT_r[:, :, ts(0, c.TT)])

            wq_sb = wqk.tile([P, c.DCH, c.DH], BF16)
            wk_sb = wqk.tile([P, c.DCH, c.DH], BF16)
            wv_sb = wqk.tile([P, c.DCH, c.DH], BF16)
            wr = {"wq": wq.rearrange("(ch p) n -> p ch n", p=P),
                  "wk": wk.rearrange("(ch p) n -> p ch n", p=P),
                  "wv": wv.rearrange("(ch p) n -> p ch n", p=P)}
            for dc in range(c.DCH):
                nc.gpsimd.dma_start(wk_sb[:, dc, :], wr["wk"][:, dc, :])
                nc.scalar.dma_start(wq_sb[:, dc, :], wr["wq"][:, dc, :])
                nc.scalar.dma_start(wv_sb[:, dc, :], wr["wv"][:, dc, :])

            # ---- constants ----
            ones_row = cst.tile([1, P], BF16)
            nc.gpsimd.dma_start(ones_row, onesin[0:1, 0:P])
            bq_sb = cst.tile([P, c.CB], F32)
            nc.scalar.dma_start(bq_sb, bq.rearrange("(cb p) -> p cb", p=P))
            bk_sb = cst.tile([P, c.CB], F32)
            nc.scalar.dma_start(bk_sb, bk.rearrange("(cb p) -> p cb", p=P))
            bv_row = cst.tile([1, c.DH], BF16)
            nc.scalar.dma_start(bv_row, bv)
            bp_row = cst.tile([1, c.DH], BF16)
            nc.scalar.dma_start(bp_row, bp)
            esel_sb = cst.tile([c.H_LOC, c.NHP * P], BF16)
            nc.gpsimd.dma_start(esel_sb, esel)
            # causal triangle for the key block AT the diagonal:
            # mask[k, j] keeps where j - k >= 0 (j = query col within window)
            mask = cst.tile([P, P], BF16)
            nc.vector.memset(mask, 1.0)
            nc.gpsimd.affine_select(
                mask, mask,
                compare_op=ALU.is_ge, fill=0.0, base=0,
                pattern=[[1, P]], channel_multiplier=-1,
            )

            # ---- persistent K^T / Q^T / V(+ones) ----
            kT = kv.tile([P, c.NHP, c.T], BF16)
            qT = kv.tile([P, c.NHP, c.T], BF16)
            v = kv.tile([P, c.T // P, c.H_LOC, c.HD + 1], BF16)
            nc.vector.memset(v[:, :, :, c.HD:c.HD + 1], 1.0)

            wp_sb = wpp.tile([P, c.GDH // P, c.DH], BF16)

            def chunks_qkv(tt):
                """QKV projection for token tile tt, as ~1.8us PE chunks."""
                xt = xts[tt]
                # K^T and Q^T: [col, token] layout
                for dst, w_sb, b_sb in (
                    (kT, wk_sb, bk_sb),
                    (qT, wq_sb, bq_sb),
                ):
                    for cb in range(c.CB):
                        pst = ps_mm.tile([P, max(c.TT, c.DH)], F32,
                                         tag="mm", name="pst")[:, :c.TT]
                        for dc in range(c.DCH):
                            nc.tensor.matmul(
                                pst,
                                w_sb[:, dc, ts(cb, P)],
                                xt[:, dc, :],
                                start=(dc == 0),
                                stop=(dc == c.DCH - 1),
                            )
                        if with_bias:
                            nc.vector.tensor_tensor(
                                dst[:, cb, ts(tt, c.TT)], pst,
                                b_sb[:, cb:cb + 1].to_broadcast((P, c.TT)),
                                ALU.add,
                            )
                        else:
                            nc.vector.tensor_copy(
                                dst[:, cb, ts(tt, c.TT)], pst)
                        yield

                # V: [token, vcol] layout (+ bias via ones-row matmul)
                for tb in range(c.TT // P):
                    gtb = tt * (c.TT // P) + tb
                    psv = ps_mm.tile([P, max(c.TT, c.DH)], F32,
                                     tag="mm", name="psv")[:, :c.DH]
                    for dc in range(c.DCH):
                        nc.tensor.matmul(
                            psv,
                            xt[:, dc, ts(tb, P)],
                            wv_sb[:, dc, :],
                            start=(dc == 0),
                            stop=(not with_bias and dc == c.DCH - 1),
                        )
                    if with_bias:
                        nc.tensor.matmul(
                            psv, ones_row[0:1, 0:P], bv_row,
                            start=False, stop=True,
                        )
                    nc.vector.tensor_copy(
                        v[:, gtb, :, 0:c.HD],
                        psv.rearrange("p (h d) -> p h d", d=c.HD),
                    )
                    yield

            def chunks_attn(qt, yt_q, yu_q, l_all):
                """Attention for query tile qt, one chunk per key-block pair.

                S^T matmuls of pair p, then the (deferred) PV of pair p-1,
                then the exp of pair p: the PE never sits directly behind
                ScalarE's exp latency.
                """
                nkb = (qt + 1) * c.QT // P
                for hp in range(c.NHP):
                    psys = [ps_y.tile([c.HD + 1, c.QT], F32, tag="psy",
                                      name=f"psy{hs}")
                            for hs in range(c.HPG)]

                    def emit_pv(kb2, blocks, pts):
                        for j in range(blocks):
                            kb = kb2 + j
                            off = max(0, kb * P - qt * c.QT)
                            for hs in range(c.HPG):
                                nc.tensor.matmul(
                                    psys[hs][:, off:],
                                    v[:, kb, hp * c.HPG + hs, :],
                                    pts[hs][:, j, off:],
                                    start=(kb == 0),
                                    stop=(kb == nkb - 1),
                                )

                    pending = None
                    for kb2 in range(0, nkb, 2):
                        blocks = min(2, nkb - kb2)
                        # S^T matmuls: the HPG heads sharing this partition
                        # group run in disjoint PE row groups (tile_position
                        # auto-derived from base_partition) => concurrent.
                        pss_l = []
                        for hs in range(c.HPG):
                            pb = hs * c.HD
                            pss = ps_s.tile([P, 2, c.QT], F32, tag="pss",
                                            name=f"pss{hs}")
                            for j in range(blocks):
                                off = max(0, (kb2 + j) * P - qt * c.QT)
                                nc.tensor.matmul(
                                    pss[:, j, off:],
                                    kT[pb:pb + c.HD, hp, ts(kb2 + j, P)],
                                    qT[pb:pb + c.HD, hp,
                                       ds(qt * c.QT + off, c.QT - off)],
                                    start=True, stop=True,
                                )
                            pss_l.append(pss)
                        if pending is not None:
                            emit_pv(*pending)
                        # exp (+ causal triangle on the diagonal blocks)
                        pt_l = []
                        for hs in range(c.HPG):
                            pt = ptp.tile([P, 2, c.QT], BF16,
                                          tag="pt", name=f"pt{hs}")
                            off0 = kb2 * P - qt * c.QT
                            if off0 < 0 and blocks == 2:
                                # fully-unmasked pair: one merged exp N=2*QT
                                nc.scalar.activation(
                                    pt.rearrange("p j q -> p (j q)"),
                                    pss_l[hs].rearrange("p j q -> p (j q)"),
                                    AF.Exp, scale=c.scale)
                            else:
                                for j in range(blocks):
                                    off = max(0, (kb2 + j) * P - qt * c.QT)
                                    nc.scalar.activation(
                                        pt[:, j, off:],
                                        pss_l[hs][:, j, off:],
                                        AF.Exp, scale=c.scale)
                            for j in range(blocks):
                                off = (kb2 + j) * P - qt * c.QT
                                if off >= 0:
                                    nc.vector.tensor_tensor(
                                        pt[:, j, off:off + P],
                                        pt[:, j, off:off + P],
                                        mask, ALU.mult,
                                    )
                            pt_l.append(pt)
                        pending = (kb2, blocks, pt_l)
                        if kb2 + 2 < nkb:
                            yield
                    emit_pv(*pending)

                    # stage unnormalized y^T + the denominators (direct
                    # partition-shifted DVE copies out of PSUM)
                    for hs in range(c.HPG):
                        pb = hs * c.HD
                        nc.vector.tensor_copy(
                            yu_q[pb:pb + c.HD, hp, :],
                            psys[hs][0:c.HD, :])
                        row = hp * c.HPG + hs
                        l_sb = lrp.tile([1, c.QT], F32, tag="lsb",
                                        name="l_sb")
                        nc.vector.tensor_copy(
                            l_sb, psys[hs][c.HD:c.HD + 1, :])
                        nc.gpsimd.dma_start(
                            l_all[row:row + 1, :], l_sb)
                    yield

                # batched 1/l for all 8 heads (DVE; consumed by the deferred
                # norm+gather chunk, which runs inside the NEXT tile's stream)
                r_all = lrp.tile([c.H_LOC, c.QT], BF16, tag="rall")
                with nc.allow_low_precision(
                    reason="1/l rounded to bf16 for the broadcast matmul"
                ):
                    nc.vector.reciprocal(r_all, l_all)
                rs[qt] = r_all

            def emit_normgather(qt, halves=1):
                """Normalize y^T and ship it (pairwise AllGather)."""
                yt_q, yu_q, r_all = yts[qt], yus[qt], rs[qt]
                for hp in range(c.NHP):
                    psr = ps_mm.tile([P, max(c.TT, c.DH)], F32,
                                     tag="mm", name="psr")[:, :c.QT]
                    nc.tensor.matmul(
                        psr, esel_sb[:, ts(hp, P)], r_all,
                        start=True, stop=True,
                    )
                    nc.vector.tensor_tensor(
                        yt_q[:, hp, :], yu_q[:, hp, :], psr, ALU.mult,
                    )
                hw_ = c.QT // halves
                y_ags = []
                for hf in range(halves):
                    y_loc = drp.tile([c.DH, hw_], BF16,
                                     tag=f"yloc{halves}", name="y_loc")
                    nc.sync.dma_start(
                        y_loc.rearrange("(hp p) t -> p hp t", p=P),
                        yt_q[:, :, ds(hf * hw_, hw_)],
                    )
                    if use_cc:
                        y_ag = drp.tile([c.GDH, hw_], BF16,
                                        tag=f"ygat{halves}", name="y_ag")
                        nc.gpsimd.collective_compute(
                            "AllGather", ALU.bypass,
                            replica_groups=groups,
                            ins=[y_loc.opt()], outs=[y_ag.opt()],
                        )
                    else:
                        y_ag = y_loc
                    y_ags.append(y_ag)
                gathers[qt] = (y_ags, halves)

            def chunks_proj(qt):
                """Column-sharded proj for query tile qt (per-128-token)."""
                y_ags, halves = gathers[qt]
                hw_ = c.QT // halves
                for hf in range(halves):
                    yag_sb = yagp.tile([P, c.GDH // P, c.QT], BF16,
                                       name="yag_sb")[:, :, :hw_]
                    nc.sync.dma_start(
                        yag_sb,
                        y_ags[hf].rearrange("(ch p) t -> p ch t", p=P),
                    )
                    for tb in range(hw_ // P):
                        gtb = hf * (hw_ // P) + tb
                        pso = ps_mm.tile([P, max(c.TT, c.DH)], F32,
                                         tag="mm", name="pso")[:, :c.DH]
                        for c2 in range(c.GDH // P):
                            nc.tensor.matmul(
                                pso,
                                yag_sb[:, c2, ts(tb, P)],
                                wp_sb[:, c2, :],
                                start=(c2 == 0),
                                stop=(not with_bias
                                      and c2 == c.GDH // P - 1),
                            )
                        if with_bias:
                            nc.tensor.matmul(
                                pso, ones_row[0:1, 0:P], bp_row,
                                start=False, stop=True,
                            )
                        osb = osbp.tile([P, c.DH], F32)
                        nc.vector.tensor_copy(osb, pso)
                        nc.gpsimd.dma_start(
                            out[ds(qt * c.QT + gtb * P, P), :], osb)
                        yield

            # ================= main interleaved schedule =================
            # Iteration tt drives attn(tt) as the main stream and weaves in:
            # norm+gather(tt-1) early (the reciprocal it needs runs on the
            # DVE during the first attention chunks), QKV(tt+1) through the
            # middle, proj(tt-1) at the back (after the gather landed).
            rs, yts, yus, gathers = {}, {}, {}, {}

            def once(fn):
                fn()
                yield

            def drive(main_gen, n_main, fillers):
                """Drain main_gen; fillers = [(f0, f1, n, gen)]: advance gen
                n times, spread over main-progress fractions [f0, f1]."""
                events = []
                for f0, f1, n, g in fillers:
                    for j in range(n):
                        events.append((f0 + (j + 0.5) * (f1 - f0) / n, g))
                events.sort(key=lambda e: e[0])
                k = 0
                i = 0
                for _ in main_gen:
                    i += 1
                    frac = i / max(1, n_main)
                    while k < len(events) and events[k][0] <= frac:
                        next(events[k][1], None)
                        k += 1
                while k < len(events):
                    next(events[k][1], None)
                    k += 1

            for tt in range(c.NTT):
                if tt + 1 < c.NTT:
                    xts[tt + 1] = xtp.tile([P, c.DCH, c.TT], BF16,
                                           name=f"xt{tt + 1}")
                    nc.sync.dma_start(
                        xts[tt + 1], xT_r[:, :, ts(tt + 1, c.TT)])
                if tt == 0:
                    for _ in chunks_qkv(0):
                        pass
                    nc.gpsimd.dma_start(
                        wp_sb, wp.rearrange("(ch p) n -> p ch n", p=P))
                yts[tt] = ytp.tile([P, c.NHP, c.QT], BF16, name="yt_q")
                yus[tt] = yup.tile([P, c.NHP, c.QT], F32, name="yu_q")
                l_all = lrp.tile([c.H_LOC, c.QT], F32, tag="lall")
                pairs = -(-((tt + 1) * c.QT // P) // 2)
                n_main = c.NHP * pairs
                nqkv = 2 * c.CB + c.TT // P
                fillers = []
                if tt >= 1:
                    fillers.append(
                        (0.10, 0.10, 1, once(lambda t=tt: emit_normgather(t - 1))))
                if tt + 1 < c.NTT:
                    fillers.append((0.15, 0.80, nqkv, chunks_qkv(tt + 1)))
                if tt >= 1:
                    nproj = c.QT // P
                    fillers.append((0.55, 0.95, nproj, chunks_proj(tt - 1)))
                drive(chunks_attn(tt, yts[tt], yus[tt], l_all),
                      n_main, fillers)

            # tail: last tile's norm + gather (split in halves so the second
            # AllGather overlaps the first half's proj) + proj
            last = c.NTT - 1
            halves = 2 if c.QT // 2 >= P else 1
            emit_normgather(last, halves)
            for _ in chunks_proj(last):
                pass

    nc.compile()
    return nc


def shard_inputs(c: Cfg, x, w_qkv, b_qkv, w_proj, b_proj, n_cores=8):
    """Full fp32 inputs -> per-core input maps (host-side marshalling).

    Matmul operands are cast to bf16 on the host; q/k biases stay fp32
    (applied via DVE adds on the f32 PSUM)."""
    D, DH = c.D, c.DH
    oc = max(128, (c.T // 128) * c.H_LOC)
    ones = np.ones((128, oc), BF16NP)
    esel = np.zeros((c.H_LOC, c.NHP * 128), BF16NP)
    for h in range(c.H_LOC):
        hp, sub = h // c.HPG, h % c.HPG
        esel[h, hp * 128 + sub * c.HD: hp * 128 + (sub + 1) * c.HD] = 1
    maps = []
    for core in range(n_cores):
        b, hh = core // c.n_groups, core % c.n_groups
        sl = slice(hh * DH, (hh + 1) * DH)
        maps.append({
            "xT": np.ascontiguousarray(x[b].T).astype(BF16NP),
            "wq": np.ascontiguousarray(
                w_qkv[:, 0 * D:1 * D][:, sl]).astype(BF16NP),
            "wk": np.ascontiguousarray(
                w_qkv[:, 1 * D:2 * D][:, sl]).astype(BF16NP),
            "wv": np.ascontiguousarray(
                w_qkv[:, 2 * D:3 * D][:, sl]).astype(BF16NP),
            "bq": np.ascontiguousarray(
                b_qkv[0 * D:1 * D][sl], dtype=np.float32),
            "bk": np.ascontiguousarray(
                b_qkv[1 * D:2 * D][sl], dtype=np.float32),
            "bv": np.ascontiguousarray(
                b_qkv[2 * D:3 * D][sl]).reshape(1, DH).astype(BF16NP),
            "wp": np.ascontiguousarray(w_proj[:, sl]).astype(BF16NP),
            "bp": np.ascontiguousarray(
                b_proj[sl]).reshape(1, DH).astype(BF16NP),
            "onesin": ones,
            "esel": esel,
        })
    return maps


def gather_outputs(c: Cfg, results, n_cores=8):
    B = n_cores // c.n_groups
    out = np.empty((B, c.T, c.GDH), dtype=np.float32)
    for core in range(n_cores):
        b, hh = core // c.n_groups, core % c.n_groups
        out[b][:, hh * c.DH:(hh + 1) * c.DH] = results[core]["out"]
    return out


_NC_CACHE: dict = {}


def kernel(**inputs) -> np.ndarray:
    from concourse.bass_utils import run_bass_kernel_spmd

    c = FULL
    n_cores = 8
    wb = bool(np.any(inputs["b_qkv"]) or np.any(inputs["b_proj"]))
    key = (c, n_cores, wb)
    if key not in _NC_CACHE:
        _NC_CACHE[key] = build_nc(c, n_cores, with_bias=wb)
    nc = _NC_CACHE[key]
    in_maps = shard_inputs(
        c, inputs["x"], inputs["w_qkv"], inputs["b_qkv"],
        inputs["w_proj"], inputs["b_proj"], n_cores,
    )
    res = run_bass_kernel_spmd(
        nc, in_maps, core_ids=list(range(n_cores)),
        trace=bool(int(os.environ.get("KERNEL_TRACE", "0"))),
    )
    kernel.last_results = res
    return gather_outputs(c, res.results, n_cores)


# revision 19
# speedup vs baseline: 1.2936x; 1.1216x over previous
"""Causal self-attention (B=4, T=2048, D=1024, H=16) on 8 trn2 NeuronCores.

Sharding: batch (4-way) x head-half (2-way tensor parallel) => 8 cores,
one uniform SPMD program (per-core differences are pure data: which batch's
x, which half of the QKV columns / proj columns each core receives).

v2 layout: the QKV projection, attention, AllGather and proj phases are
interleaved at query-tile granularity so every engine has work throughout
the kernel (v1 ran them as four sequential phases; the PE idled during the
ScalarE-bound softmax phase long enough for the HAM clock gate to hold the
array at half clock for ~250us of the run).

Per core (batch b, head-half hh, 8 local heads), all matmul operands bf16:
  for tt in 0..3 (token/query tiles of 512):
    1. QKV(tt): q^T/k^T in [qkv_col, token] layout; v in [token, vcol+1]
       layout (ones column => PV matmul accumulates the softmax denominator
       l in PSUM row HD).
    2. attention(qt=tt): per head-partition-group, stream key-block PAIRS
       (2x128): S^T matmuls (2 heads concurrent in disjoint PE row groups)
       -> one merged exp per pair on ScalarE (off-diagonal pairs N=1024)
       -> PV deferred one pair (software pipeline) so the PE never waits
       on the exp latency.
    3. normalization: l rows of all 8 heads batched into one DVE
       reciprocal, broadcast across partitions via a K=8 esel matmul.
    4. ship y^T: pairwise AllGather (bf16) with the partner core; overlaps
       the next tile's QKV matmuls.
    5. proj(qt-1): column-sharded, emitted one tile behind so the gather
       has a full QKV+attention window to land.
"""

import os
import sys
from dataclasses import dataclass

import ml_dtypes
import numpy as np

sys.path.insert(0, "/opt/trn_rl_repo")

import concourse.mybir as mybir  # noqa: E402
import concourse.tile as tile  # noqa: E402
from concourse import bacc  # noqa: E402
from concourse.bass import ds, ts  # noqa: E402

P = 128
F32 = mybir.dt.float32
BF16 = mybir.dt.bfloat16
AF = mybir.ActivationFunctionType
ALU = mybir.AluOpType
BF16NP = ml_dtypes.bfloat16


@dataclass(frozen=True)
class Cfg:
    T: int = 2048          # sequence length
    D: int = 1024          # model dim (QKV contraction dim)
    H_LOC: int = 8         # heads per core
    HD: int = 64           # head dim
    TT: int = 512          # token tile width in the QKV phase
    QT: int = 512          # query tile width in the attention phase
    n_groups: int = 2      # cores sharing a batch (pairwise AllGather)
    scale: float = 64 ** -0.5

    @property
    def DH(self):          # local head dims (y^T rows contributed per core)
        return self.H_LOC * self.HD

    @property
    def GDH(self):         # proj contraction dim (= model dim)
        return self.n_groups * self.DH

    @property
    def DCH(self):
        return self.D // P

    @property
    def NHP(self):         # 128-partition groups of local head dims
        return self.DH // P

    @property
    def HPG(self):         # heads per 128-partition group
        return P // self.HD

    @property
    def NTT(self):
        return self.T // self.TT

    @property
    def NQT(self):
        return self.T // self.QT

    @property
    def CB(self):          # 128-wide column blocks of the local q/k cols
        return self.DH // P


FULL = Cfg()


def build_nc(c: Cfg, n_cores: int = 8, with_bias: bool = True):
    """Build the (uniform SPMD) Bass program for one core."""
    assert c.T % c.TT == 0 and c.T % c.QT == 0 and c.QT % P == 0
    assert c.D % P == 0 and c.DH % P == 0 and c.TT % P == 0
    assert c.TT == c.QT, "QKV token tiles and query tiles must align"
    use_cc = c.n_groups > 1

    nc = bacc.Bacc(
        "TRN2", target_bir_lowering=False, debug=False, num_devices=n_cores
    )
    xT = nc.dram_tensor("xT", [c.D, c.T], BF16, kind="ExternalInput").ap()
    wq = nc.dram_tensor("wq", [c.D, c.DH], BF16, kind="ExternalInput").ap()
    wk = nc.dram_tensor("wk", [c.D, c.DH], BF16, kind="ExternalInput").ap()
    wv = nc.dram_tensor("wv", [c.D, c.DH], BF16, kind="ExternalInput").ap()
    bq = nc.dram_tensor("bq", [c.DH], F32, kind="ExternalInput").ap()
    bk = nc.dram_tensor("bk", [c.DH], F32, kind="ExternalInput").ap()
    bv = nc.dram_tensor("bv", [1, c.DH], BF16, kind="ExternalInput").ap()
    wp = nc.dram_tensor("wp", [c.GDH, c.DH], BF16, kind="ExternalInput").ap()
    bp = nc.dram_tensor("bp", [1, c.DH], BF16, kind="ExternalInput").ap()
    oc = max(P, (c.T // P) * c.H_LOC)
    onesin = nc.dram_tensor("onesin", [P, oc], BF16, kind="ExternalInput").ap()
    esel = nc.dram_tensor("esel", [c.H_LOC, c.NHP * P], BF16,
                          kind="ExternalInput").ap()
    out = nc.dram_tensor("out", [c.T, c.DH], F32, kind="ExternalOutput").ap()

    groups = [[g * c.n_groups + i for i in range(c.n_groups)]
              for g in range(max(1, n_cores // c.n_groups))]

    with tile.TileContext(nc) as tc:
        with (
            tc.tile_pool(name="const", bufs=1) as cst,
            tc.tile_pool(name="kv", bufs=1) as kv,
            tc.tile_pool(name="wqkv", bufs=1) as wqk,
            tc.tile_pool(name="xt", bufs=2) as xtp,
            tc.tile_pool(name="wproj", bufs=1) as wpp,
            tc.tile_pool(name="pt", bufs=4) as ptp,
            tc.tile_pool(name="yt", bufs=2) as ytp,
            tc.tile_pool(name="yu", bufs=1) as yup,
            tc.tile_pool(name="lr", bufs=2) as lrp,
            tc.tile_pool(name="yag", bufs=1) as yagp,
            tc.tile_pool(name="osb", bufs=2) as osbp,
            tc.tile_pool(name="ps_mm", bufs=2, space="PSUM") as ps_mm,
            tc.tile_pool(name="ps_s", bufs=2, space="PSUM") as ps_s,
            tc.tile_pool(name="ps_y", bufs=2, space="PSUM") as ps_y,
            tc.tile_pool(name="dram", bufs=2, space="DRAM") as drp,
        ):
            # ---- first input tile + weights (DMA-order: critical first) ----
            xT_r = xT.rearrange("(ch p) t -> p ch t", p=P)
            xts = [None] * c.NTT
            xts[0] = xtp.tile([P, c.DCH, c.TT], BF16, name="xt0")
            for dc2 in range(0, c.DCH, 2):
                w2 = min(2, c.DCH - dc2)
                nc.sync.dma_start(xts[0][:, dc2:dc2 + w2, :],
                                  xT_r[:, dc2:dc2 + w2, ts(0, c.TT)])

            wq_sb = wqk.tile([P, c.DCH, c.DH], BF16)
            wk_sb = wqk.tile([P, c.DCH, c.DH], BF16)
            wv_sb = wqk.tile([P, c.DCH, c.DH], BF16)
            wr = {"wq": wq.rearrange("(ch p) n -> p ch n", p=P),
                  "wk": wk.rearrange("(ch p) n -> p ch n", p=P),
                  "wv": wv.rearrange("(ch p) n -> p ch n", p=P)}
            for dc in range(c.DCH):
                nc.gpsimd.dma_start(wk_sb[:, dc, :], wr["wk"][:, dc, :])
                nc.scalar.dma_start(wq_sb[:, dc, :], wr["wq"][:, dc, :])
                nc.scalar.dma_start(wv_sb[:, dc, :], wr["wv"][:, dc, :])

            # ---- constants ----
            ones_row = cst.tile([1, P], BF16)
            nc.gpsimd.dma_start(ones_row, onesin[0:1, 0:P])
            bq_sb = cst.tile([P, c.CB], F32)
            nc.scalar.dma_start(bq_sb, bq.rearrange("(cb p) -> p cb", p=P))
            bk_sb = cst.tile([P, c.CB], F32)
            nc.scalar.dma_start(bk_sb, bk.rearrange("(cb p) -> p cb", p=P))
            bv_row = cst.tile([1, c.DH], BF16)
            nc.scalar.dma_start(bv_row, bv)
            bp_row = cst.tile([1, c.DH], BF16)
            nc.scalar.dma_start(bp_row, bp)
            # esel halves: separate SBUF tiles so both start at partition 0
            # (DVE/PE base partitions must be 32-aligned)
            n_hv = 2 if c.NHP % 2 == 0 else 1
            hpn = c.NHP // n_hv           # partition groups per norm half
            hh = c.H_LOC // n_hv          # heads per norm half
            esel_sbs = []
            for hv in range(n_hv):
                esel_sb = cst.tile([hh, c.NHP * P], BF16,
                                   name=f"esel_sb{hv}")
                nc.gpsimd.dma_start(esel_sb, esel[hv * hh:(hv + 1) * hh, :])
                esel_sbs.append(esel_sb)
            # causal triangle for the key block AT the diagonal:
            # mask[k, j] keeps where j - k >= 0 (j = query col within window)
            mask = cst.tile([P, P], BF16)
            nc.vector.memset(mask, 1.0)
            nc.gpsimd.affine_select(
                mask, mask,
                compare_op=ALU.is_ge, fill=0.0, base=0,
                pattern=[[1, P]], channel_multiplier=-1,
            )

            # ---- persistent K^T / Q^T / V(+ones) ----
            kT = kv.tile([P, c.NHP, c.T], BF16)
            qT = kv.tile([P, c.NHP, c.T], BF16)
            v = kv.tile([P, c.T // P, c.H_LOC, c.HD + 1], BF16)
            nc.vector.memset(v[:, :, :, c.HD:c.HD + 1], 1.0)

            wp_sb = wpp.tile([P, c.GDH // P, c.DH], BF16)

            def chunks_qkv(tt):
                """QKV projection for token tile tt, as ~1.8us PE chunks."""
                xt = xts[tt]
                # K^T and Q^T: [col, token] layout
                for dst, w_sb, b_sb in (
                    (kT, wk_sb, bk_sb),
                    (qT, wq_sb, bq_sb),
                ):
                    for cb in range(c.CB):
                        pst = ps_mm.tile([P, max(c.TT, c.DH)], F32,
                                         tag="mm", name="pst")[:, :c.TT]
                        for dc in range(c.DCH):
                            nc.tensor.matmul(
                                pst,
                                w_sb[:, dc, ts(cb, P)],
                                xt[:, dc, :],
                                start=(dc == 0),
                                stop=(dc == c.DCH - 1),
                            )
                        if with_bias:
                            nc.vector.tensor_tensor(
                                dst[:, cb, ts(tt, c.TT)], pst,
                                b_sb[:, cb:cb + 1].to_broadcast((P, c.TT)),
                                ALU.add,
                            )
                        else:
                            nc.vector.tensor_copy(
                                dst[:, cb, ts(tt, c.TT)], pst)
                        yield

                # V: [token, vcol] layout (+ bias via ones-row matmul)
                for tb in range(c.TT // P):
                    gtb = tt * (c.TT // P) + tb
                    psv = ps_mm.tile([P, max(c.TT, c.DH)], F32,
                                     tag="mm", name="psv")[:, :c.DH]
                    for dc in range(c.DCH):
                        nc.tensor.matmul(
                            psv,
                            xt[:, dc, ts(tb, P)],
                            wv_sb[:, dc, :],
                            start=(dc == 0),
                            stop=(not with_bias and dc == c.DCH - 1),
                        )
                    if with_bias:
                        nc.tensor.matmul(
                            psv, ones_row[0:1, 0:P], bv_row,
                            start=False, stop=True,
                        )
                    nc.vector.tensor_copy(
                        v[:, gtb, :, 0:c.HD],
                        psv.rearrange("p (h d) -> p h d", d=c.HD),
                    )
                    yield

            def chunks_attn(qt, yt_q, yu_q, l_halves):
                """Attention for query tile qt, one chunk per key-block pair.

                S^T matmuls of pair p, then the (deferred) PV of pair p-1,
                then the exp of pair p: the PE never sits directly behind
                ScalarE's exp latency.
                """
                nkb = (qt + 1) * c.QT // P
                for hp in range(c.NHP):
                    l_all = l_halves[hp // hpn]
                    psys = [ps_y.tile([c.HD + 1, c.QT], F32, tag="psy",
                                      name=f"psy{hs}")
                            for hs in range(c.HPG)]

                    def emit_pv(kb2, blocks, pts):
                        for j in range(blocks):
                            kb = kb2 + j
                            off = max(0, kb * P - qt * c.QT)
                            for hs in range(c.HPG):
                                nc.tensor.matmul(
                                    psys[hs][:, off:],
                                    v[:, kb, hp * c.HPG + hs, :],
                                    pts[hs][:, j, off:],
                                    start=(kb == 0),
                                    stop=(kb == nkb - 1),
                                )

                    pending = None
                    for kb2 in range(0, nkb, 2):
                        blocks = min(2, nkb - kb2)
                        # S^T matmuls: the HPG heads sharing this partition
                        # group run in disjoint PE row groups (tile_position
                        # auto-derived from base_partition) => concurrent.
                        pss_l = []
                        for hs in range(c.HPG):
                            pb = hs * c.HD
                            pss = ps_s.tile([P, 2, c.QT], F32, tag="pss",
                                            name=f"pss{hs}")
                            for j in range(blocks):
                                off = max(0, (kb2 + j) * P - qt * c.QT)
                                nc.tensor.matmul(
                                    pss[:, j, off:],
                                    kT[pb:pb + c.HD, hp, ts(kb2 + j, P)],
                                    qT[pb:pb + c.HD, hp,
                                       ds(qt * c.QT + off, c.QT - off)],
                                    start=True, stop=True,
                                )
                            pss_l.append(pss)
                        if pending is not None:
                            emit_pv(*pending)
                        # exp (+ causal triangle on the diagonal blocks)
                        pt_l = []
                        for hs in range(c.HPG):
                            pt = ptp.tile([P, 2, c.QT], BF16,
                                          tag="pt", name=f"pt{hs}")
                            off0 = kb2 * P - qt * c.QT
                            if off0 < 0 and blocks == 2:
                                # fully-unmasked pair: one merged exp N=2*QT
                                nc.scalar.activation(
                                    pt.rearrange("p j q -> p (j q)"),
                                    pss_l[hs].rearrange("p j q -> p (j q)"),
                                    AF.Exp, scale=c.scale)
                            else:
                                for j in range(blocks):
                                    off = max(0, (kb2 + j) * P - qt * c.QT)
                                    nc.scalar.activation(
                                        pt[:, j, off:],
                                        pss_l[hs][:, j, off:],
                                        AF.Exp, scale=c.scale)
                            for j in range(blocks):
                                off = (kb2 + j) * P - qt * c.QT
                                if off >= 0:
                                    nc.vector.tensor_tensor(
                                        pt[:, j, off:off + P],
                                        pt[:, j, off:off + P],
                                        mask, ALU.mult,
                                    )
                            pt_l.append(pt)
                        pending = (kb2, blocks, pt_l)
                        if kb2 + 2 < nkb:
                            yield
                    emit_pv(*pending)

                    # stage unnormalized y^T + the denominators (direct
                    # partition-shifted DVE copies out of PSUM)
                    for hs in range(c.HPG):
                        pb = hs * c.HD
                        nc.vector.tensor_copy(
                            yu_q[pb:pb + c.HD, hp, :],
                            psys[hs][0:c.HD, :])
                        row = (hp % hpn) * c.HPG + hs
                        l_sb = lrp.tile([1, c.QT], F32, tag="lsb",
                                        name="l_sb")
                        nc.vector.tensor_copy(
                            l_sb, psys[hs][c.HD:c.HD + 1, :])
                        nc.gpsimd.dma_start(
                            l_all[row:row + 1, :], l_sb)
                    yield

            def emit_normgather(qt, hv, l_half):
                """Normalize one head-half of y^T and ship it (pairwise
                AllGather of DH/n_hv dims x QT tokens)."""
                yt_q, yu_q = yts[qt], yus[qt]
                # 1/l: fast custom-DVE reciprocal (~18 correct bits), then
                # round to bf16 for the K=hh broadcast matmul
                r_f = lrp.tile([hh, c.QT], F32, tag="rf", name="r_f")
                nc.vector.reciprocal_approx_fast(r_f, l_half)
                r_x = lrp.tile([hh, c.QT], BF16, tag="rall", name="r_x")
                nc.vector.tensor_copy(r_x, r_f)
                for hp in range(hv * hpn, (hv + 1) * hpn):
                    psr = ps_mm.tile([P, max(c.TT, c.DH)], F32,
                                     tag="mm", name="psr")[:, :c.QT]
                    nc.tensor.matmul(
                        psr, esel_sbs[hv][:, ts(hp, P)], r_x,
                        start=True, stop=True,
                    )
                    nc.vector.tensor_tensor(
                        yt_q[:, hp, :], yu_q[:, hp, :], psr, ALU.mult,
                    )
                y_loc = drp.tile([c.DH // n_hv, c.QT], BF16,
                                 tag=f"yloc{hv}", name="y_loc")
                nc.sync.dma_start(
                    y_loc.rearrange("(hp p) t -> p hp t", p=P),
                    yt_q[:, hv * hpn:(hv + 1) * hpn, :],
                )
                if use_cc:
                    y_ag = drp.tile([c.GDH // n_hv, c.QT], BF16,
                                    tag=f"ygat{hv}", name="y_ag")
                    nc.gpsimd.collective_compute(
                        "AllGather", ALU.bypass,
                        replica_groups=groups,
                        ins=[y_loc.opt()], outs=[y_ag.opt()],
                    )
                else:
                    y_ag = y_loc
                gathers[(qt, hv)] = y_ag

            def chunks_proj(qt):
                """Column-sharded proj for query tile qt (per-128-token).

                With n_hv=2 the gathered y^T arrives as two dim-half tiles:
                global contraction block c2 -> (half, block-within-half).
                """
                yag_sbs = []
                for hv in range(n_hv):
                    yag_sb = yagp.tile([P, c.GDH // (P * n_hv), c.QT], BF16,
                                       tag=f"yag{hv}", name="yag_sb")
                    nc.sync.dma_start(
                        yag_sb,
                        gathers[(qt, hv)].rearrange(
                            "(ch p) t -> p ch t", p=P),
                    )
                    yag_sbs.append(yag_sb)
                for tb in range(c.QT // P):
                    pso = ps_mm.tile([P, max(c.TT, c.DH)], F32,
                                     tag="mm", name="pso")[:, :c.DH]
                    for c2 in range(c.GDH // P):
                        hv = (c2 % c.NHP) // hpn
                        idx = (c2 // c.NHP) * hpn + (c2 % c.NHP) % hpn
                        nc.tensor.matmul(
                            pso,
                            yag_sbs[hv][:, idx, ts(tb, P)],
                            wp_sb[:, c2, :],
                            start=(c2 == 0),
                            stop=(not with_bias
                                  and c2 == c.GDH // P - 1),
                        )
                    if with_bias:
                        nc.tensor.matmul(
                            pso, ones_row[0:1, 0:P], bp_row,
                            start=False, stop=True,
                        )
                    osb = osbp.tile([P, c.DH], F32)
                    nc.vector.tensor_copy(osb, pso)
                    nc.gpsimd.dma_start(
                        out[ds(qt * c.QT + tb * P, P), :], osb)
                    yield

            # ================= main interleaved schedule =================
            # Iteration tt drives attn(tt) as the main stream and weaves in:
            # norm+gather of tt's FIRST head-half just past attn(tt)'s
            # midpoint (its heads are finished then), norm+gather of the
            # SECOND half early in attn(tt+1), QKV(tt+1) through the middle,
            # proj(tt-1) at the back (once both its gathers landed).
            yts, yus, gathers = {}, {}, {}

            if use_cc:
                # tiny warm-up AllGather: absorbs the ~10us first-collective
                # setup cost while QKV(0) runs
                cc_w_in = drp.tile([P, 16], BF16, name="cc_w_in")
                cc_w_out = drp.tile([P * c.n_groups, 16], BF16,
                                    name="cc_w_out")
                nc.gpsimd.dma_start(cc_w_in, mask[:, 0:16])
                nc.gpsimd.collective_compute(
                    "AllGather", ALU.bypass, replica_groups=groups,
                    ins=[cc_w_in.opt()], outs=[cc_w_out.opt()],
                )

            def once(fn):
                fn()
                yield

            def drive(main_gen, n_main, fillers):
                """Drain main_gen; fillers = [(f0, f1, n, gen)]: advance gen
                n times, spread over main-progress fractions [f0, f1]."""
                events = []
                for f0, f1, n, g in fillers:
                    for j in range(n):
                        events.append((f0 + (j + 0.5) * (f1 - f0) / n, g))
                events.sort(key=lambda e: e[0])
                k = 0
                i = 0
                for _ in main_gen:
                    i += 1
                    frac = i / max(1, n_main)
                    while k < len(events) and events[k][0] <= frac:
                        next(events[k][1], None)
                        k += 1
                while k < len(events):
                    next(events[k][1], None)
                    k += 1

            for tt in range(c.NTT):
                if tt + 1 < c.NTT:
                    xts[tt + 1] = xtp.tile([P, c.DCH, c.TT], BF16,
                                           name=f"xt{tt + 1}")
                    nc.sync.dma_start(
                        xts[tt + 1], xT_r[:, :, ts(tt + 1, c.TT)])
                if tt == 0:
                    for _ in chunks_qkv(0):
                        pass
                    nc.gpsimd.dma_start(
                        wp_sb, wp.rearrange("(ch p) n -> p ch n", p=P))
                yts[tt] = ytp.tile([P, c.NHP, c.QT], BF16, name="yt_q")
                yus[tt] = yup.tile([P, c.NHP, c.QT], F32, name="yu_q")
                l_halves = [lrp.tile([hh, c.QT], F32, tag=f"l{hv}",
                                     name=f"l_half{hv}")
                            for hv in range(n_hv)]
                pairs = -(-((tt + 1) * c.QT // P) // 2)
                n_main = c.NHP * pairs
                nqkv = 2 * c.CB + c.TT // P
                fillers = []
                if tt >= 1:
                    # previous tile's second head-half: normalize + gather
                    fillers.append(
                        (0.10, 0.10, 1,
                         once(lambda t=tt, lh=lhs_prev: emit_normgather(
                             t - 1, n_hv - 1, lh[n_hv - 1]))))
                if tt + 1 < c.NTT:
                    fillers.append((0.15, 0.80, nqkv, chunks_qkv(tt + 1)))
                if n_hv == 2:
                    # this tile's first head-half is final at frac 0.5
                    fillers.append(
                        (0.56, 0.56, 1,
                         once(lambda t=tt, lh=l_halves: emit_normgather(
                             t, 0, lh[0]))))
                if tt >= 1:
                    nproj = c.QT // P
                    fillers.append((0.55, 0.95, nproj, chunks_proj(tt - 1)))
                drive(chunks_attn(tt, yts[tt], yus[tt], l_halves),
                      n_main, fillers)
                lhs_prev = l_halves

            # tail: last tile's second head-half + its proj
            last = c.NTT - 1
            emit_normgather(last, n_hv - 1, lhs_prev[n_hv - 1])
            for _ in chunks_proj(last):
                pass

    nc.compile()
    return nc


def shard_inputs(c: Cfg, x, w_qkv, b_qkv, w_proj, b_proj, n_cores=8):
    """Full fp32 inputs -> per-core input maps (host-side marshalling).

    Matmul operands are cast to bf16 on the host; q/k biases stay fp32
    (applied via DVE adds on the f32 PSUM)."""
    D, DH = c.D, c.DH
    oc = max(128, (c.T // 128) * c.H_LOC)
    ones = np.ones((128, oc), BF16NP)
    esel = np.zeros((c.H_LOC, c.NHP * 128), BF16NP)
    for h in range(c.H_LOC):
        hp, sub = h // c.HPG, h % c.HPG
        esel[h, hp * 128 + sub * c.HD: hp * 128 + (sub + 1) * c.HD] = 1
    maps = []
    for core in range(n_cores):
        b, hh = core // c.n_groups, core % c.n_groups
        sl = slice(hh * DH, (hh + 1) * DH)
        maps.append({
            "xT": np.ascontiguousarray(x[b].T).astype(BF16NP),
            "wq": np.ascontiguousarray(
                w_qkv[:, 0 * D:1 * D][:, sl]).astype(BF16NP),
            "wk": np.ascontiguousarray(
                w_qkv[:, 1 * D:2 * D][:, sl]).astype(BF16NP),
            "wv": np.ascontiguousarray(
                w_qkv[:, 2 * D:3 * D][:, sl]).astype(BF16NP),
            "bq": np.ascontiguousarray(
                b_qkv[0 * D:1 * D][sl], dtype=np.float32),
            "bk": np.ascontiguousarray(
                b_qkv[1 * D:2 * D][sl], dtype=np.float32),
            "bv": np.ascontiguousarray(
                b_qkv[2 * D:3 * D][sl]).reshape(1, DH).astype(BF16NP),
            "wp": np.ascontiguousarray(w_proj[:, sl]).astype(BF16NP),
            "bp": np.ascontiguousarray(
                b_proj[sl]).reshape(1, DH).astype(BF16NP),
            "onesin": ones,
            "esel": esel,
        })
    return maps


def gather_outputs(c: Cfg, results, n_cores=8):
    B = n_cores // c.n_groups
    out = np.empty((B, c.T, c.GDH), dtype=np.float32)
    for core in range(n_cores):
        b, hh = core // c.n_groups, core % c.n_groups
        out[b][:, hh * c.DH:(hh + 1) * c.DH] = results[core]["out"]
    return out


_NC_CACHE: dict = {}


def kernel(**inputs) -> np.ndarray:
    from concourse.bass_utils import run_bass_kernel_spmd

    c = FULL
    n_cores = 8
    wb = bool(np.any(inputs["b_qkv"]) or np.any(inputs["b_proj"]))
    key = (c, n_cores, wb)
    if key not in _NC_CACHE:
        _NC_CACHE[key] = build_nc(c, n_cores, with_bias=wb)
    nc = _NC_CACHE[key]
    in_maps = shard_inputs(
        c, inputs["x"], inputs["w_qkv"], inputs["b_qkv"],
        inputs["w_proj"], inputs["b_proj"], n_cores,
    )
    res = run_bass_kernel_spmd(
        nc, in_maps, core_ids=list(range(n_cores)),
        trace=bool(int(os.environ.get("KERNEL_TRACE", "0"))),
    )
    kernel.last_results = res
    return gather_outputs(c, res.results, n_cores)
